# revision 23
# baseline (speedup 1.0000x reference)
"""Multi-head attention (softmax over the QUERY axis) on 8 TRN2 NeuronCores.

Problem shapes: Q [T=1024, B=8, D=256]; per-head full-width projections
Wq/Wk/Wv [H=8, E=512, D=256]; Wo [D=256, H*E=4096].

Sharding: data-parallel over batch B — core b computes all H heads for
batch b. No collectives; the host re-stacks per-core outputs along B.

Per-core layout (scores matmuls fp8-DoubleRow, rest bf16, accum fp32):
  qT[e,t] = fp8(Wq_h @ Q_b^T * s*SQ + bq*s*SQ)
  kT[e,t] = fp8(Wk_h @ Q_b^T * SK + bk*SK)
  vT[e,t] = bf16(Wv_h @ Q_b^T + bv)
  AT[s,t] =  kT^T-blocks x qT  (DoubleRow fp8: 2 e-blocks per instr)
  E[s,t]  =  exp(AT/(SQ*SK))   (softmax axis t = free axis; |logits|<=~6)
  l[s]    =  row-sum of E (fused accum_out of the Exp activation)
  W2[s,d] = (vT^T-blocks x Wo_h^T) / l[s]   -- AV and out-proj FUSED:
  out[d,t] += W2^T-blocks x E     out = sum_s E[s,t] (sum_e V'[s,e] Wo[d,e])
                                  contracting the TxT attention into d=256
                                  width instead of e=512 halves that matmul.
Output leaves the device [d, t]-transposed; the host transposes back.
"""

import sys

sys.path.insert(0, "/opt/trn_rl_repo")

from contextlib import ExitStack

import ml_dtypes
import numpy as np

import concourse.bass as bass
import concourse.tile as tile
from concourse.tile import add_dep_helper
from concourse import bacc, bass_utils, mybir

T, B, D, H, E = 1024, 8, 256, 8, 512
N_CORES = 8

F32 = mybir.dt.float32
BF16 = mybir.dt.bfloat16
FP8 = mybir.dt.float8e4
DR = mybir.MatmulPerfMode.DoubleRow
AF = mybir.ActivationFunctionType

# fp8 range tuning: q' scaled by SQ, k by SK (folded into weights host-side);
# the Exp activation divides the scores PSUM by SQ*SK.
SQ, SK = 32.0, 8.0


def build_nc(t=T, d=D, h=H, e=E):
    """Build the per-core SPMD program. Returns a compiled Bacc."""
    TC = t // 512   # t chunks (512-wide matmul free dim)
    SB = t // 128   # s blocks (keys == queries length)
    EB = e // 128   # e blocks
    DC = d // 128   # d chunks (contraction for projections)
    DB = d // 128   # d blocks (partition tiles of the transposed out)

    nc = bacc.Bacc("TRN2", target_bir_lowering=False, debug=False)

    # All big inputs arrive pre-arranged partition-major ([128, free...])
    # so every load is a clean 2D DMA with one contiguous row per partition.
    qt_d = nc.dram_tensor("qt", [128, DC, t], BF16, kind="ExternalInput").ap()
    wqt_d = nc.dram_tensor("wqt", [h, 128, DC, e], BF16, kind="ExternalInput").ap()
    wkt_d = nc.dram_tensor("wkt", [h, 128, DC, e], BF16, kind="ExternalInput").ap()
    wvt_d = nc.dram_tensor("wvt", [h, 128, DC, e], BF16, kind="ExternalInput").ap()
    wot_d = nc.dram_tensor("wot", [h, 128, EB, d], BF16, kind="ExternalInput").ap()
    bq_d = nc.dram_tensor("bqs", [128, h, EB], F32, kind="ExternalInput").ap()
    bk_d = nc.dram_tensor("bks", [128, h, EB], F32, kind="ExternalInput").ap()
    bv_d = nc.dram_tensor("bvs", [128, h, EB], F32, kind="ExternalInput").ap()
    bo_d = nc.dram_tensor("bos", [128, DB], F32, kind="ExternalInput").ap()
    out_d = nc.dram_tensor("out", [d, t], F32, kind="ExternalOutput").ap()

    with tile.TileContext(nc) as tc, ExitStack() as ctx:
        consts = ctx.enter_context(tc.tile_pool(name="consts", bufs=1))
        wpool = ctx.enter_context(tc.tile_pool(name="wpool", bufs=2))
        hpool = ctx.enter_context(tc.tile_pool(name="hpool", bufs=2))
        spool = ctx.enter_context(tc.tile_pool(name="spool", bufs=2))
        at_pool = ctx.enter_context(tc.tile_pool(name="at_pool", bufs=3, space="PSUM"))
        mm_pool = ctx.enter_context(tc.tile_pool(name="mm_pool", bufs=5, space="PSUM"))

        # ---- persistent loads -------------------------------------------
        # Q^T: the dc0 chunk split in quarters across DMA queues so the
        # first projection matmul's inputs land ASAP.
        qt_sb = consts.tile([128, DC, t], BF16)
        for qtr in range(4):
            qsl = slice(qtr * (t // 4), (qtr + 1) * (t // 4))
            nc.sync.dma_start(out=qt_sb[:, 0, qsl], in_=qt_d[:, 0, qsl])
        bq_sb = consts.tile([128, h, EB], F32)
        nc.sync.dma_start(out=bq_sb, in_=bq_d)
        bk_sb = consts.tile([128, h, EB], F32)
        nc.sync.dma_start(out=bk_sb, in_=bk_d)
        bv_sb = consts.tile([128, h, EB], F32)
        nc.sync.dma_start(out=bv_sb, in_=bv_d)
        bo_sb = consts.tile([128, DB], F32)
        nc.gpsimd.dma_start(out=bo_sb, in_=bo_d)
        out_acc = consts.tile([128, DB, t], F32)
        out_r = out_d.rearrange("(db p) t -> p db t", p=128)

        # ---- PE warm-up: dummy matmuls during the initial DMA wait so the
        # HAM clock-gate ramps before real work lands ----------------------
        scratch = consts.tile([128, 640], BF16)
        nc.vector.memset(scratch, 0.0)
        ps_w = mm_pool.tile([128, 512], F32, tag="mm")
        for _ in range(10):
            nc.tensor.matmul(
                ps_w, scratch[:, :128], scratch[:, 128:640], start=True, stop=True
            )

        # Per-head SBUF tile handles, filled by emit_loads/emit_proj.
        wq_t, wk_t, wv_t, wo_t = {}, {}, {}, {}
        qT_t, kT_t, vT_t = {}, {}, {}
        gated = []  # head-0 bulk loads deferred past the first matmul
        first_mm = [None]

        def emit_loads(hh):
            wq_sb = wpool.tile([128, DC, e], BF16, name="wq")
            for dc in range(DC):
                nc.sync.dma_start(out=wq_sb[:, dc, :], in_=wqt_d[hh, :, dc, :])
            if hh == 0:
                # rest of Q^T after the critical wq chunks
                for qtr in range(4):
                    qsl = slice(qtr * (t // 4), (qtr + 1) * (t // 4))
                    nc.sync.dma_start(out=qt_sb[:, 1, qsl], in_=qt_d[:, 1, qsl])
            wk_sb = wpool.tile([128, DC, e], BF16, name="wk")
            for dc in range(DC):
                nc.sync.dma_start(out=wk_sb[:, dc, :], in_=wkt_d[hh, :, dc, :])
            wv_sb = wpool.tile([128, DC, e], BF16, name="wv")
            g1 = nc.sync.dma_start(out=wv_sb, in_=wvt_d[hh])
            wo_sb = wpool.tile([128, EB, d], BF16, name="wo")
            g2 = nc.sync.dma_start(out=wo_sb, in_=wot_d[hh])
            if hh == 0:
                gated.extend([g1, g2])
            wq_t[hh], wk_t[hh], wv_t[hh], wo_t[hh] = wq_sb, wk_sb, wv_sb, wo_sb

        def emit_proj(hh):
            """q/k (fp8 out) and v (bf16) projections, transposed [e,t]."""
            wq_sb, wk_sb, wv_sb = wq_t[hh], wk_t[hh], wv_t[hh]
            qT = hpool.tile([128, EB, t], FP8, name="qT")
            kT = hpool.tile([128, EB, t], FP8, name="kT")
            vT = hpool.tile([128, EB, t], BF16, name="vT")
            for eb in range(EB):
                for tch in range(TC):
                    tsl = slice(tch * 512, (tch + 1) * 512)
                    ps_q = mm_pool.tile([128, 512], F32, tag="mm", name="ps_q")
                    for dc in range(DC):
                        mm = nc.tensor.matmul(
                            ps_q,
                            wq_sb[:, dc, eb * 128 : (eb + 1) * 128],
                            qt_sb[:, dc, tsl],
                            start=(dc == 0),
                            stop=(dc == DC - 1),
                        )
                        if first_mm[0] is None:
                            first_mm[0] = mm
                    nc.vector.tensor_scalar_add(
                        qT[:, eb, tsl], ps_q, bq_sb[:, hh, eb : eb + 1]
                    )
            if hh == 0:
                for g in gated:
                    add_dep_helper(
                        g.ins, first_mm[0].ins,
                        reason="defer bulk load past cold start",
                    )
            for eb in range(EB):
                for tch in range(TC):
                    tsl = slice(tch * 512, (tch + 1) * 512)
                    ps_k = mm_pool.tile([128, 512], F32, tag="mm", name="ps_k")
                    for dc in range(DC):
                        nc.tensor.matmul(
                            ps_k,
                            wk_sb[:, dc, eb * 128 : (eb + 1) * 128],
                            qt_sb[:, dc, tsl],
                            start=(dc == 0),
                            stop=(dc == DC - 1),
                        )
                    nc.scalar.activation(
                        kT[:, eb, tsl],
                        ps_k,
                        AF.Identity,
                        bias=bk_sb[:, hh, eb : eb + 1],
                    )
            for eb in range(EB):
                for tch in range(TC):
                    tsl = slice(tch * 512, (tch + 1) * 512)
                    ps_v = mm_pool.tile([128, 512], F32, tag="mm", name="ps_v")
                    for dc in range(DC):
                        nc.tensor.matmul(
                            ps_v,
                            wv_sb[:, dc, eb * 128 : (eb + 1) * 128],
                            qt_sb[:, dc, tsl],
                            start=(dc == 0),
                            stop=(dc == DC - 1),
                        )
                    nc.vector.tensor_scalar_add(
                        vT[:, eb, tsl], ps_v, bv_sb[:, hh, eb : eb + 1]
                    )
            qT_t[hh], kT_t[hh], vT_t[hh] = qT, kT, vT

        def emit_scores(hh):
            """fp8-DoubleRow scores -> exp (scaled) -> rowsum -> 1/l."""
            qT, kT = qT_t[hh], kT_t[hh]
            Ex = hpool.tile([128, SB, t], BF16, name="Ex")
            lsum2 = spool.tile([128, SB, TC], F32, name="lsum2")
            lsum = spool.tile([128, SB], F32, name="lsum")
            rr = spool.tile([128, SB], F32, name="rr")
            for sb in range(SB):
                ssl = slice(sb * 128, (sb + 1) * 128)
                for tch in range(TC):
                    tsl = slice(tch * 512, (tch + 1) * 512)
                    at = at_pool.tile([128, 512], F32, tag="at", name="at")
                    for ebp in range(EB // 2):
                        nc.tensor.matmul(
                            at,
                            kT[:, 2 * ebp : 2 * ebp + 2, ssl],
                            qT[:, 2 * ebp : 2 * ebp + 2, tsl],
                            start=(ebp == 0),
                            stop=(ebp == EB // 2 - 1),
                            perf_mode=DR,
                        )
                    nc.scalar.activation(
                        Ex[:, sb, tsl],
                        at,
                        AF.Exp,
                        scale=float(1.0 / (SQ * SK)),
                        accum_out=lsum2[:, sb, tch : tch + 1],
                    )
                if TC == 1:
                    nc.vector.reciprocal(rr[:, sb : sb + 1], lsum2[:, sb, 0:1])
                else:
                    nc.vector.reduce_sum(
                        lsum[:, sb : sb + 1],
                        lsum2[:, sb, :],
                        axis=mybir.AxisListType.X,
                    )
                    nc.vector.reciprocal(rr[:, sb : sb + 1], lsum[:, sb : sb + 1])
            return Ex, rr

        def emit_w2_out(hh, Ex, rr):
            """W2 = (vT^T x Wo^T)/l, then out[d,t] += W2^T x E."""
            wo_sb = wo_t[hh]
            vT = vT_t[hh]
            W2 = hpool.tile([128, SB, d], BF16, name="W2")
            for sb in range(SB):
                ssl = slice(sb * 128, (sb + 1) * 128)
                ps_2 = mm_pool.tile([128, 512], F32, tag="mm", name="ps_2")
                for eb in range(EB):
                    nc.tensor.matmul(
                        ps_2[:, :d],
                        vT[:, eb, ssl],
                        wo_sb[:, eb, :],
                        start=(eb == 0),
                        stop=(eb == EB - 1),
                    )
                nc.vector.tensor_scalar_mul(
                    W2[:, sb, :], ps_2[:, :d], rr[:, sb : sb + 1]
                )
            for db in range(DB):
                dsl = slice(db * 128, (db + 1) * 128)
                for tch in range(TC):
                    tsl = slice(tch * 512, (tch + 1) * 512)
                    ps_f = mm_pool.tile([128, 512], F32, tag="mm", name="ps_f")
                    for sc in range(SB):
                        nc.tensor.matmul(
                            ps_f,
                            W2[:, sc, dsl],
                            Ex[:, sc, tsl],
                            start=(sc == 0),
                            stop=(sc == SB - 1),
                        )
                    if hh == 0:
                        nc.vector.tensor_scalar_add(
                            out_acc[:, db, tsl], ps_f, bo_sb[:, db : db + 1]
                        )
                    else:
                        nc.vector.tensor_add(
                            out_acc[:, db, tsl], out_acc[:, db, tsl], ps_f
                        )
                    if hh == h - 1:
                        # two half-width stores on separate DMA rings so the
                        # final transfer isn't single-ring bound
                        for hf in range(2):
                            psl = slice(
                                tch * 512 + hf * 256, tch * 512 + (hf + 1) * 256
                            )
                            nc.sync.dma_start(
                                out=out_r[:, db, psl], in_=out_acc[:, db, psl]
                            )

        # ---- head loop (tile scheduler handles cross-phase overlap) -----
        for hh in range(h):
            emit_loads(hh)
            emit_proj(hh)
            ex_rr = emit_scores(hh)
            emit_w2_out(hh, *ex_rr)

    nc.compile()
    return nc


_NC_CACHE = {}


def _get_nc(shape_key):
    if shape_key not in _NC_CACHE:
        _NC_CACHE[shape_key] = build_nc(*shape_key)
    return _NC_CACHE[shape_key]


def _pmajor(a, last):
    """[..., C*128, last] -> [..., 128, C, last] partition-major layout."""
    lead = a.shape[:-2]
    c = a.shape[-2] // 128
    return np.ascontiguousarray(
        a.reshape(*lead, c, 128, last).swapaxes(-3, -2)
    )


def _perpart(v, scale=1.0):
    """[h, e] bias -> [128, h, EB] per-partition layout."""
    h, e = v.shape
    return np.ascontiguousarray(
        (np.asarray(v, np.float32) * scale).reshape(h, -1, 128).transpose(2, 0, 1)
    )


def _prep_inputs(Q, Wq, bq, Wk, bk, Wv, bv, Wo, bo):
    t, b, d = Q.shape
    h, e, _ = Wq.shape
    s = np.float32(1.0 / np.sqrt(e))
    bf = ml_dtypes.bfloat16
    Q = np.asarray(Q, np.float32)
    # [B, 128, DC, T] partition-major Q^T per batch
    qt_all = _pmajor(Q.transpose(1, 2, 0).astype(bf), t)
    wqt = _pmajor(
        (np.asarray(Wq, np.float32).transpose(0, 2, 1) * (s * SQ)).astype(bf), e
    )
    wkt = _pmajor(
        (np.asarray(Wk, np.float32).transpose(0, 2, 1) * SK).astype(bf), e
    )
    wvt = _pmajor(np.asarray(Wv, np.float32).transpose(0, 2, 1).astype(bf), e)
    wot = _pmajor(np.asarray(Wo, np.float32).T.reshape(h, e, d).astype(bf), d)
    shared = {
        "wqt": wqt,
        "wkt": wkt,
        "wvt": wvt,
        "wot": wot,
        "bqs": _perpart(bq, s * SQ),
        "bks": _perpart(bk, SK),
        "bvs": _perpart(bv),
        "bos": np.ascontiguousarray(
            np.asarray(bo, np.float32).reshape(-1, 128).T
        ),
    }
    in_maps = [
        {"qt": np.ascontiguousarray(qt_all[bb]), **shared} for bb in range(b)
    ]
    return in_maps, (t, d, h, e)


def kernel(Q, Wq, bq, Wk, bk, Wv, bv, Wo, bo, _trace=False):
    in_maps, (t, d, h, e) = _prep_inputs(Q, Wq, bq, Wk, bk, Wv, bv, Wo, bo)
    nc = _get_nc((t, d, h, e))
    res = bass_utils.run_bass_kernel_spmd(
        nc, in_maps, core_ids=list(range(len(in_maps))), trace=_trace
    )
    # device output is [d, t]; transpose back per core
    out = np.stack(
        [res.results[b]["out"].T for b in range(len(in_maps))], axis=1
    )
    if _trace:
        kernel.last_results = res
    return np.ascontiguousarray(out.astype(np.float32))


# revision 24
# speedup vs baseline: 1.2027x; 1.2027x over previous
"""Multi-head attention (softmax over the QUERY axis) on 8 TRN2 NeuronCores.

Problem shapes: Q [T=1024, B=8, D=256]; per-head full-width projections
Wq/Wk/Wv [H=8, E=512, D=256]; Wo [D=256, H*E=4096].

Sharding: data-parallel over batch B — core b computes all H heads for
batch b. No collectives; the host re-stacks per-core outputs along B.

Per-core layout (scores matmuls fp8-DoubleRow, rest bf16, accum fp32):
  qT[e,t] = fp8(Wq_h @ Q_b^T * s*SQ + bq*s*SQ)
  kT[e,t] = fp8(Wk_h @ Q_b^T * SK + bk*SK)
  vT[e,t] = bf16(Wv_h @ Q_b^T + bv)
  AT[s,t] =  kT^T-blocks x qT  (DoubleRow fp8: 2 e-blocks per instr)
  E[s,t]  =  exp(AT/(SQ*SK))   (softmax axis t = free axis; |logits|<=~6)
  l[s]    =  row-sum of E (fused accum_out of the Exp activation)
  W2[s,d] = (vT^T-blocks x Wo_h^T) / l[s]   -- AV and out-proj FUSED:
  out[d,t] += W2^T-blocks x E     out = sum_s E[s,t] (sum_e V'[s,e] Wo[d,e])
                                  contracting the TxT attention into d=256
                                  width instead of e=512 halves that matmul.
Output leaves the device [d, t]-transposed; the host transposes back.
"""

import sys

sys.path.insert(0, "/opt/trn_rl_repo")

from contextlib import ExitStack

import ml_dtypes
import numpy as np

import concourse.bass as bass
import concourse.tile as tile
from concourse.tile import add_dep_helper
from concourse import bacc, bass_utils, mybir

T, B, D, H, E = 1024, 8, 256, 8, 512
N_CORES = 8

F32 = mybir.dt.float32
BF16 = mybir.dt.bfloat16
FP8 = mybir.dt.float8e4
DR = mybir.MatmulPerfMode.DoubleRow
AF = mybir.ActivationFunctionType

# fp8 range tuning: q' scaled by SQ, k by SK (folded into weights host-side);
# the Exp activation divides the scores PSUM by SQ*SK.
SQ, SK = 32.0, 8.0


def build_nc(t=T, d=D, h=H, e=E):
    """Build the per-core SPMD program. Returns a compiled Bacc."""
    TC = t // 512   # t chunks (512-wide matmul free dim)
    SB = t // 128   # s blocks (keys == queries length)
    EB = e // 128   # e blocks
    DC = d // 128   # d chunks (contraction for projections)
    DB = d // 128   # d blocks (partition tiles of the transposed out)

    nc = bacc.Bacc("TRN2", target_bir_lowering=False, debug=False)

    # All big inputs arrive pre-arranged partition-major ([128, free...])
    # so every load is a clean 2D DMA with one contiguous row per partition.
    qt_d = nc.dram_tensor("qt", [128, DC, t], BF16, kind="ExternalInput").ap()
    wqt_d = nc.dram_tensor("wqt", [h, 128, DC, e], BF16, kind="ExternalInput").ap()
    wkt_d = nc.dram_tensor("wkt", [h, 128, DC, e], BF16, kind="ExternalInput").ap()
    wvt_d = nc.dram_tensor("wvt", [h, 128, DC, e], BF16, kind="ExternalInput").ap()
    wot_d = nc.dram_tensor("wot", [h, 128, EB, d], BF16, kind="ExternalInput").ap()
    bq_d = nc.dram_tensor("bqs", [128, h, EB], F32, kind="ExternalInput").ap()
    bk_d = nc.dram_tensor("bks", [128, h, EB], F32, kind="ExternalInput").ap()
    bv_d = nc.dram_tensor("bvs", [128, h, EB], F32, kind="ExternalInput").ap()
    bo_d = nc.dram_tensor("bos", [128, DB], F32, kind="ExternalInput").ap()
    out_d = nc.dram_tensor("out", [d, t], F32, kind="ExternalOutput").ap()

    with tile.TileContext(nc) as tc, ExitStack() as ctx:
        consts = ctx.enter_context(tc.tile_pool(name="consts", bufs=1))
        wpool = ctx.enter_context(tc.tile_pool(name="wpool", bufs=2))
        hpool = ctx.enter_context(tc.tile_pool(name="hpool", bufs=2))
        spool = ctx.enter_context(tc.tile_pool(name="spool", bufs=2))
        at_pool = ctx.enter_context(tc.tile_pool(name="at_pool", bufs=3, space="PSUM"))
        mm_pool = ctx.enter_context(tc.tile_pool(name="mm_pool", bufs=5, space="PSUM"))

        # ---- persistent loads -------------------------------------------
        # Q^T: the dc0 chunk split in quarters across DMA queues so the
        # first projection matmul's inputs land ASAP.
        qt_sb = consts.tile([128, DC, t], BF16)
        for qtr in range(4):
            qsl = slice(qtr * (t // 4), (qtr + 1) * (t // 4))
            nc.sync.dma_start(out=qt_sb[:, 0, qsl], in_=qt_d[:, 0, qsl])
        bq_sb = consts.tile([128, h, EB], F32)
        nc.sync.dma_start(out=bq_sb, in_=bq_d)
        bk_sb = consts.tile([128, h, EB], F32)
        nc.sync.dma_start(out=bk_sb, in_=bk_d)
        bv_sb = consts.tile([128, h, EB], F32)
        nc.sync.dma_start(out=bv_sb, in_=bv_d)
        bo_sb = consts.tile([128, DB], F32)
        nc.gpsimd.dma_start(out=bo_sb, in_=bo_d)
        out_acc = consts.tile([128, DB, t], F32)
        out_r = out_d.rearrange("(db p) t -> p db t", p=128)

        # ---- PE warm-up: dummy matmuls during the initial DMA wait so the
        # HAM clock-gate ramps before real work lands ----------------------
        scratch = consts.tile([128, 640], BF16)
        nc.vector.memset(scratch, 0.0)
        ps_w = mm_pool.tile([128, 512], F32, tag="mm")
        for _ in range(10):
            nc.tensor.matmul(
                ps_w, scratch[:, :128], scratch[:, 128:640], start=True, stop=True
            )

        # Per-head SBUF tile handles, filled by emit_loads/emit_proj.
        wq_t, wk_t, wv_t, wo_t = {}, {}, {}, {}
        qT_t, kT_t, vT_t = {}, {}, {}
        gated = []  # head-0 bulk loads deferred past the first matmul
        first_mm = [None]

        def emit_loads(hh):
            wq_sb = wpool.tile([128, DC, e], BF16, name="wq")
            for dc in range(DC):
                nc.sync.dma_start(out=wq_sb[:, dc, :], in_=wqt_d[hh, :, dc, :])
            if hh == 0:
                # rest of Q^T after the critical wq chunks
                for qtr in range(4):
                    qsl = slice(qtr * (t // 4), (qtr + 1) * (t // 4))
                    nc.sync.dma_start(out=qt_sb[:, 1, qsl], in_=qt_d[:, 1, qsl])
            wk_sb = wpool.tile([128, DC, e], BF16, name="wk")
            for dc in range(DC):
                nc.sync.dma_start(out=wk_sb[:, dc, :], in_=wkt_d[hh, :, dc, :])
            wv_sb = wpool.tile([128, DC, e], BF16, name="wv")
            g1 = nc.sync.dma_start(out=wv_sb, in_=wvt_d[hh])
            wo_sb = wpool.tile([128, EB, d], BF16, name="wo")
            g2 = nc.sync.dma_start(out=wo_sb, in_=wot_d[hh])
            if hh == 0:
                gated.extend([g1, g2])
            wq_t[hh], wk_t[hh], wv_t[hh], wo_t[hh] = wq_sb, wk_sb, wv_sb, wo_sb

        def emit_proj(hh):
            """q/k (fp8 out) and v (bf16) projections, transposed [e,t]."""
            wq_sb, wk_sb, wv_sb = wq_t[hh], wk_t[hh], wv_t[hh]
            qT = hpool.tile([128, EB, t], FP8, name="qT")
            kT = hpool.tile([128, EB, t], FP8, name="kT")
            vT = hpool.tile([128, EB, t], BF16, name="vT")
            for eb in range(EB):
                for tch in range(TC):
                    tsl = slice(tch * 512, (tch + 1) * 512)
                    ps_q = mm_pool.tile([128, 512], F32, tag="mm", name="ps_q")
                    for dc in range(DC):
                        mm = nc.tensor.matmul(
                            ps_q,
                            wq_sb[:, dc, eb * 128 : (eb + 1) * 128],
                            qt_sb[:, dc, tsl],
                            start=(dc == 0),
                            stop=(dc == DC - 1),
                        )
                        if first_mm[0] is None:
                            first_mm[0] = mm
                    nc.vector.tensor_scalar_add(
                        qT[:, eb, tsl], ps_q, bq_sb[:, hh, eb : eb + 1]
                    )
            if hh == 0:
                for g in gated:
                    add_dep_helper(
                        g.ins, first_mm[0].ins,
                        reason="defer bulk load past cold start",
                    )
            for eb in range(EB):
                for tch in range(TC):
                    tsl = slice(tch * 512, (tch + 1) * 512)
                    ps_k = mm_pool.tile([128, 512], F32, tag="mm", name="ps_k")
                    for dc in range(DC):
                        nc.tensor.matmul(
                            ps_k,
                            wk_sb[:, dc, eb * 128 : (eb + 1) * 128],
                            qt_sb[:, dc, tsl],
                            start=(dc == 0),
                            stop=(dc == DC - 1),
                        )
                    nc.scalar.activation(
                        kT[:, eb, tsl],
                        ps_k,
                        AF.Identity,
                        bias=bk_sb[:, hh, eb : eb + 1],
                    )
            for eb in range(EB):
                for tch in range(TC):
                    tsl = slice(tch * 512, (tch + 1) * 512)
                    ps_v = mm_pool.tile([128, 512], F32, tag="mm", name="ps_v")
                    for dc in range(DC):
                        nc.tensor.matmul(
                            ps_v,
                            wv_sb[:, dc, eb * 128 : (eb + 1) * 128],
                            qt_sb[:, dc, tsl],
                            start=(dc == 0),
                            stop=(dc == DC - 1),
                        )
                    nc.vector.tensor_scalar_add(
                        vT[:, eb, tsl], ps_v, bv_sb[:, hh, eb : eb + 1]
                    )
            qT_t[hh], kT_t[hh], vT_t[hh] = qT, kT, vT

        def emit_scores(hh):
            """fp8-DoubleRow scores -> exp (scaled) -> rowsum -> 1/l."""
            qT, kT = qT_t[hh], kT_t[hh]
            Ex = hpool.tile([128, SB, t], BF16, name="Ex")
            lsum2 = spool.tile([128, SB, TC], F32, name="lsum2")
            lsum = spool.tile([128, SB], F32, name="lsum")
            rr = spool.tile([128, SB], F32, name="rr")
            for sb in range(SB):
                ssl = slice(sb * 128, (sb + 1) * 128)
                for tch in range(TC):
                    tsl = slice(tch * 512, (tch + 1) * 512)
                    at = at_pool.tile([128, 512], F32, tag="at", name="at")
                    for ebp in range(EB // 2):
                        nc.tensor.matmul(
                            at,
                            kT[:, 2 * ebp : 2 * ebp + 2, ssl],
                            qT[:, 2 * ebp : 2 * ebp + 2, tsl],
                            start=(ebp == 0),
                            stop=(ebp == EB // 2 - 1),
                            perf_mode=DR,
                        )
                    nc.scalar.activation(
                        Ex[:, sb, tsl],
                        at,
                        AF.Exp,
                        scale=float(1.0 / (SQ * SK)),
                        accum_out=lsum2[:, sb, tch : tch + 1],
                    )
                if TC == 1:
                    nc.vector.reciprocal(rr[:, sb : sb + 1], lsum2[:, sb, 0:1])
                else:
                    nc.vector.reduce_sum(
                        lsum[:, sb : sb + 1],
                        lsum2[:, sb, :],
                        axis=mybir.AxisListType.X,
                    )
                    nc.vector.reciprocal(rr[:, sb : sb + 1], lsum[:, sb : sb + 1])
            return Ex, rr

        def emit_w2_out(hh, Ex, rr):
            """W2 = (vT^T x Wo^T)/l, then out[d,t] += W2^T x E."""
            wo_sb = wo_t[hh]
            vT = vT_t[hh]
            W2 = hpool.tile([128, SB, d], BF16, name="W2")
            for sb in range(SB):
                ssl = slice(sb * 128, (sb + 1) * 128)
                ps_2 = mm_pool.tile([128, 512], F32, tag="mm", name="ps_2")
                for eb in range(EB):
                    nc.tensor.matmul(
                        ps_2[:, :d],
                        vT[:, eb, ssl],
                        wo_sb[:, eb, :],
                        start=(eb == 0),
                        stop=(eb == EB - 1),
                    )
                nc.vector.tensor_scalar_mul(
                    W2[:, sb, :], ps_2[:, :d], rr[:, sb : sb + 1]
                )
            for db in range(DB):
                dsl = slice(db * 128, (db + 1) * 128)
                for tch in range(TC):
                    tsl = slice(tch * 512, (tch + 1) * 512)
                    ps_f = mm_pool.tile([128, 512], F32, tag="mm", name="ps_f")
                    for sc in range(SB):
                        nc.tensor.matmul(
                            ps_f,
                            W2[:, sc, dsl],
                            Ex[:, sc, tsl],
                            start=(sc == 0),
                            stop=(sc == SB - 1),
                        )
                    if hh == 0:
                        nc.vector.tensor_scalar_add(
                            out_acc[:, db, tsl], ps_f, bo_sb[:, db : db + 1]
                        )
                    else:
                        nc.vector.tensor_add(
                            out_acc[:, db, tsl], out_acc[:, db, tsl], ps_f
                        )
                    if hh == h - 1:
                        nc.sync.dma_start(
                            out=out_r[:, db, tsl], in_=out_acc[:, db, tsl]
                        )

        # ---- head loop (tile scheduler handles cross-phase overlap) -----
        for hh in range(h):
            emit_loads(hh)
            emit_proj(hh)
            ex_rr = emit_scores(hh)
            emit_w2_out(hh, *ex_rr)

    nc.compile()
    return nc


_NC_CACHE = {}


def _get_nc(shape_key):
    if shape_key not in _NC_CACHE:
        _NC_CACHE[shape_key] = build_nc(*shape_key)
    return _NC_CACHE[shape_key]


def _pmajor(a, last):
    """[..., C*128, last] -> [..., 128, C, last] partition-major layout."""
    lead = a.shape[:-2]
    c = a.shape[-2] // 128
    return np.ascontiguousarray(
        a.reshape(*lead, c, 128, last).swapaxes(-3, -2)
    )


def _perpart(v, scale=1.0):
    """[h, e] bias -> [128, h, EB] per-partition layout."""
    h, e = v.shape
    return np.ascontiguousarray(
        (np.asarray(v, np.float32) * scale).reshape(h, -1, 128).transpose(2, 0, 1)
    )


def _prep_inputs(Q, Wq, bq, Wk, bk, Wv, bv, Wo, bo):
    t, b, d = Q.shape
    h, e, _ = Wq.shape
    s = np.float32(1.0 / np.sqrt(e))
    bf = ml_dtypes.bfloat16
    Q = np.asarray(Q, np.float32)
    # [B, 128, DC, T] partition-major Q^T per batch
    qt_all = _pmajor(Q.transpose(1, 2, 0).astype(bf), t)
    wqt = _pmajor(
        (np.asarray(Wq, np.float32).transpose(0, 2, 1) * (s * SQ)).astype(bf), e
    )
    wkt = _pmajor(
        (np.asarray(Wk, np.float32).transpose(0, 2, 1) * SK).astype(bf), e
    )
    wvt = _pmajor(np.asarray(Wv, np.float32).transpose(0, 2, 1).astype(bf), e)
    wot = _pmajor(np.asarray(Wo, np.float32).T.reshape(h, e, d).astype(bf), d)
    shared = {
        "wqt": wqt,
        "wkt": wkt,
        "wvt": wvt,
        "wot": wot,
        "bqs": _perpart(bq, s * SQ),
        "bks": _perpart(bk, SK),
        "bvs": _perpart(bv),
        "bos": np.ascontiguousarray(
            np.asarray(bo, np.float32).reshape(-1, 128).T
        ),
    }
    in_maps = [
        {"qt": np.ascontiguousarray(qt_all[bb]), **shared} for bb in range(b)
    ]
    return in_maps, (t, d, h, e)


def kernel(Q, Wq, bq, Wk, bk, Wv, bv, Wo, bo, _trace=False):
    in_maps, (t, d, h, e) = _prep_inputs(Q, Wq, bq, Wk, bk, Wv, bv, Wo, bo)
    nc = _get_nc((t, d, h, e))
    res = bass_utils.run_bass_kernel_spmd(
        nc, in_maps, core_ids=list(range(len(in_maps))), trace=_trace
    )
    # device output is [d, t]; transpose back per core
    out = np.stack(
        [res.results[b]["out"].T for b in range(len(in_maps))], axis=1
    )
    if _trace:
        kernel.last_results = res
    return np.ascontiguousarray(out.astype(np.float32))


# revision 26
# speedup vs baseline: 1.3122x; 1.0910x over previous
"""Multi-head attention (softmax over the QUERY axis) on 8 TRN2 NeuronCores.

Problem shapes: Q [T=1024, B=8, D=256]; per-head full-width projections
Wq/Wk/Wv [H=8, E=512, D=256]; Wo [D=256, H*E=4096].

Sharding: data-parallel over batch B — core b computes all H heads for
batch b. No collectives; the host re-stacks per-core outputs along B.

Both big einsum chains are factored through D=256 (< E=512), since
scores and output have rank <= D:
  scores: S = (Wq Q)^T (Wk Q) = Q^T M Q with M = s*Wq^T Wk  [D,D, host]
  output: out = sum_s E[s,t] W2[s,d] with W2 = (v @ Wo_h)/l
Softmax over the QUERY axis t makes all per-key-row constants cancel in
E/l, so the bq bias terms drop exactly; the surviving per-query term
s*(Wq^T bk)@Q_t is added to the scores as a rank-1 matmul (ones x
broadcast row, host-precomputed). All matmuls bf16, accum fp32.

Per-core, per-head:
  MQ[d',t] = M_h @ Q_b^T            (8 matmuls)
  AT[s,t]  = Q MQ + 1 x wqg         (16 tiles x (2+1) matmuls)
  E[s,t]   = exp(AT); l[s] = row-sum (fused accum_out)
  vT[e,t]  = Wv_h @ Q_b^T + bv
  W2[s,d]  = (vT^T-blocks x Wo_h^T) / l[s]
  out[d,t] += W2^T-blocks x E
Output leaves the device [d, t]-transposed; the host transposes back.
"""

import sys

sys.path.insert(0, "/opt/trn_rl_repo")

from contextlib import ExitStack

import ml_dtypes
import numpy as np

import concourse.bass as bass
import concourse.tile as tile
from concourse.tile import add_dep_helper
from concourse import bacc, bass_utils, mybir

T, B, D, H, E = 1024, 8, 256, 8, 512
N_CORES = 8

F32 = mybir.dt.float32
BF16 = mybir.dt.bfloat16
AF = mybir.ActivationFunctionType


def _bcast(ap_row, parts):
    """Partition-broadcast a [1, n] DRAM AP to [parts, n] (step-0 partition)."""
    return bass.AP(
        tensor=ap_row.tensor,
        offset=ap_row.offset,
        ap=[[0, parts], list(ap_row.ap[-1])],
    )


def build_nc(t=T, d=D, h=H, e=E):
    """Build the per-core SPMD program. Returns a compiled Bacc."""
    TC = t // 512   # t chunks (512-wide matmul free dim)
    SB = t // 128   # s blocks (keys == queries length)
    EB = e // 128   # e blocks
    DC = d // 128   # d chunks (contraction for projections / scores)
    DB = d // 128   # d blocks (partition tiles of the transposed out)

    nc = bacc.Bacc("TRN2", target_bir_lowering=False, debug=False)

    # All big inputs arrive pre-arranged partition-major ([128, free...])
    # so every load is a clean 2D DMA with one contiguous row per partition.
    qt_d = nc.dram_tensor("qt", [128, DC, t], BF16, kind="ExternalInput").ap()
    mqt_d = nc.dram_tensor("mqt", [h, 128, DC, d], BF16, kind="ExternalInput").ap()
    wvt_d = nc.dram_tensor("wvt", [h, 128, DC, e], BF16, kind="ExternalInput").ap()
    wot_d = nc.dram_tensor("wot", [h, 128, EB, d], BF16, kind="ExternalInput").ap()
    wqg_d = nc.dram_tensor("wqg", [h, t], BF16, kind="ExternalInput").ap()
    bv_d = nc.dram_tensor("bvs", [128, h, EB], F32, kind="ExternalInput").ap()
    bo_d = nc.dram_tensor("bos", [128, DB], F32, kind="ExternalInput").ap()
    out_d = nc.dram_tensor("out", [d, t], F32, kind="ExternalOutput").ap()

    with tile.TileContext(nc) as tc, ExitStack() as ctx:
        consts = ctx.enter_context(tc.tile_pool(name="consts", bufs=1))
        wpool = ctx.enter_context(tc.tile_pool(name="wpool", bufs=2))
        hpool = ctx.enter_context(tc.tile_pool(name="hpool", bufs=2))
        spool = ctx.enter_context(tc.tile_pool(name="spool", bufs=2))
        at_pool = ctx.enter_context(tc.tile_pool(name="at_pool", bufs=3, space="PSUM"))
        mm_pool = ctx.enter_context(tc.tile_pool(name="mm_pool", bufs=5, space="PSUM"))

        # ---- persistent loads -------------------------------------------
        # Q^T: the dc0 chunk split in quarters across DMA queues so the
        # first matmul's inputs land ASAP.
        qt_sb = consts.tile([128, DC, t], BF16)
        for qtr in range(4):
            qsl = slice(qtr * (t // 4), (qtr + 1) * (t // 4))
            nc.sync.dma_start(out=qt_sb[:, 0, qsl], in_=qt_d[:, 0, qsl])
        bv_sb = consts.tile([128, h, EB], F32)
        nc.sync.dma_start(out=bv_sb, in_=bv_d)
        bo_sb = consts.tile([128, DB], F32)
        nc.gpsimd.dma_start(out=bo_sb, in_=bo_d)
        out_acc = consts.tile([128, DB, t], F32)
        out_r = out_d.rearrange("(db p) t -> p db t", p=128)

        # ones column for the rank-1 bias-term matmul (host divides by 128)
        ones_sb = consts.tile([128, 128], BF16)
        nc.vector.memset(ones_sb, 1.0)

        # ---- PE warm-up: dummy matmuls during the initial DMA wait so the
        # HAM clock-gate ramps before real work lands ----------------------
        scratch = consts.tile([128, 640], BF16)
        nc.vector.memset(scratch, 0.0)
        ps_w = mm_pool.tile([128, 512], F32, tag="mm")
        for _ in range(10):
            nc.tensor.matmul(
                ps_w, scratch[:, :128], scratch[:, 128:640], start=True, stop=True
            )

        # Per-head SBUF tile handles, filled by emit_loads/emit_mqv.
        mq_t, wv_t, wo_t, wqg_t = {}, {}, {}, {}
        MQ_t, vT_t = {}, {}
        gated = []  # head-0 bulk loads deferred past the first matmul
        first_mm = [None]

        def emit_loads(hh):
            mq_sb = wpool.tile([128, DC, d], BF16, name="mq")
            for dc in range(DC):
                nc.sync.dma_start(out=mq_sb[:, dc, :], in_=mqt_d[hh, :, dc, :])
            if hh == 0:
                # rest of Q^T after the critical M chunks
                for qtr in range(4):
                    qsl = slice(qtr * (t // 4), (qtr + 1) * (t // 4))
                    nc.sync.dma_start(out=qt_sb[:, 1, qsl], in_=qt_d[:, 1, qsl])
            wv_sb = wpool.tile([128, DC, e], BF16, name="wv")
            g1 = nc.sync.dma_start(out=wv_sb, in_=wvt_d[hh])
            wo_sb = wpool.tile([128, EB, d], BF16, name="wo")
            g2 = nc.sync.dma_start(out=wo_sb, in_=wot_d[hh])
            wqg_bc = wpool.tile([128, t], BF16, name="wqg")
            g3 = nc.gpsimd.dma_start(out=wqg_bc, in_=_bcast(wqg_d[hh][None, :], 128))
            if hh == 0:
                gated.extend([g1, g2, g3])
            mq_t[hh], wv_t[hh], wo_t[hh], wqg_t[hh] = mq_sb, wv_sb, wo_sb, wqg_bc

        def emit_mqv(hh):
            """MQ = M_h @ Q^T (bf16 out) and vT = Wv_h @ Q^T + bv."""
            mq_sb, wv_sb = mq_t[hh], wv_t[hh]
            MQ = hpool.tile([128, DC, t], BF16, name="MQ")
            vT = hpool.tile([128, EB, t], BF16, name="vT")
            for db in range(DB):
                for tch in range(TC):
                    tsl = slice(tch * 512, (tch + 1) * 512)
                    ps_m = mm_pool.tile([128, 512], F32, tag="mm", name="ps_m")
                    for dc in range(DC):
                        mm = nc.tensor.matmul(
                            ps_m,
                            mq_sb[:, dc, db * 128 : (db + 1) * 128],
                            qt_sb[:, dc, tsl],
                            start=(dc == 0),
                            stop=(dc == DC - 1),
                        )
                        if first_mm[0] is None:
                            first_mm[0] = mm
                    nc.vector.tensor_copy(MQ[:, db, tsl], ps_m)
            if hh == 0:
                for g in gated:
                    add_dep_helper(
                        g.ins, first_mm[0].ins,
                        reason="defer bulk load past cold start",
                    )
            for eb in range(EB):
                for tch in range(TC):
                    tsl = slice(tch * 512, (tch + 1) * 512)
                    ps_v = mm_pool.tile([128, 512], F32, tag="mm", name="ps_v")
                    for dc in range(DC):
                        nc.tensor.matmul(
                            ps_v,
                            wv_sb[:, dc, eb * 128 : (eb + 1) * 128],
                            qt_sb[:, dc, tsl],
                            start=(dc == 0),
                            stop=(dc == DC - 1),
                        )
                    nc.scalar.activation(
                        vT[:, eb, tsl],
                        ps_v,
                        AF.Identity,
                        bias=bv_sb[:, hh, eb : eb + 1],
                    )
            MQ_t[hh], vT_t[hh] = MQ, vT

        def emit_scores(hh):
            """AT = Q^T M Q + rank-1 bias term -> exp -> rowsum -> 1/l."""
            MQ, wqg_bc = MQ_t[hh], wqg_t[hh]
            Ex = hpool.tile([128, SB, t], BF16, name="Ex")
            lsum2 = spool.tile([128, SB, TC], F32, name="lsum2")
            lsum = spool.tile([128, SB], F32, name="lsum")
            rr = spool.tile([128, SB], F32, name="rr")
            for sb in range(SB):
                ssl = slice(sb * 128, (sb + 1) * 128)
                for tch in range(TC):
                    tsl = slice(tch * 512, (tch + 1) * 512)
                    at = at_pool.tile([128, 512], F32, tag="at", name="at")
                    nc.tensor.matmul(
                        at, ones_sb, wqg_bc[:, tsl], start=True, stop=False
                    )
                    for dc in range(DC):
                        nc.tensor.matmul(
                            at,
                            qt_sb[:, dc, ssl],
                            MQ[:, dc, tsl],
                            start=False,
                            stop=(dc == DC - 1),
                        )
                    nc.scalar.activation(
                        Ex[:, sb, tsl],
                        at,
                        AF.Exp,
                        accum_out=lsum2[:, sb, tch : tch + 1],
                    )
                if TC == 1:
                    nc.vector.reciprocal(rr[:, sb : sb + 1], lsum2[:, sb, 0:1])
                else:
                    nc.vector.reduce_sum(
                        lsum[:, sb : sb + 1],
                        lsum2[:, sb, :],
                        axis=mybir.AxisListType.X,
                    )
                    nc.vector.reciprocal(rr[:, sb : sb + 1], lsum[:, sb : sb + 1])
            return Ex, rr

        def emit_w2_out(hh, Ex, rr):
            """W2 = (vT^T x Wo^T)/l, then out[d,t] += W2^T x E."""
            wo_sb = wo_t[hh]
            vT = vT_t[hh]
            W2 = hpool.tile([128, SB, d], BF16, name="W2")
            for sb in range(SB):
                ssl = slice(sb * 128, (sb + 1) * 128)
                ps_2 = mm_pool.tile([128, 512], F32, tag="mm", name="ps_2")
                for eb in range(EB):
                    nc.tensor.matmul(
                        ps_2[:, :d],
                        vT[:, eb, ssl],
                        wo_sb[:, eb, :],
                        start=(eb == 0),
                        stop=(eb == EB - 1),
                    )
                nc.vector.tensor_scalar_mul(
                    W2[:, sb, :], ps_2[:, :d], rr[:, sb : sb + 1]
                )
            for db in range(DB):
                dsl = slice(db * 128, (db + 1) * 128)
                for tch in range(TC):
                    tsl = slice(tch * 512, (tch + 1) * 512)
                    ps_f = mm_pool.tile([128, 512], F32, tag="mm", name="ps_f")
                    for sc in range(SB):
                        nc.tensor.matmul(
                            ps_f,
                            W2[:, sc, dsl],
                            Ex[:, sc, tsl],
                            start=(sc == 0),
                            stop=(sc == SB - 1),
                        )
                    if hh == 0:
                        nc.vector.tensor_scalar_add(
                            out_acc[:, db, tsl], ps_f, bo_sb[:, db : db + 1]
                        )
                    else:
                        nc.vector.tensor_add(
                            out_acc[:, db, tsl], out_acc[:, db, tsl], ps_f
                        )
                    if hh == h - 1:
                        nc.sync.dma_start(
                            out=out_r[:, db, tsl], in_=out_acc[:, db, tsl]
                        )

        # ---- head loop (tile scheduler handles cross-phase overlap) -----
        for hh in range(h):
            emit_loads(hh)
            emit_mqv(hh)
            ex_rr = emit_scores(hh)
            emit_w2_out(hh, *ex_rr)

    nc.compile()
    return nc


_NC_CACHE = {}


def _get_nc(shape_key):
    if shape_key not in _NC_CACHE:
        _NC_CACHE[shape_key] = build_nc(*shape_key)
    return _NC_CACHE[shape_key]


def _pmajor(a, last):
    """[..., C*128, last] -> [..., 128, C, last] partition-major layout."""
    lead = a.shape[:-2]
    c = a.shape[-2] // 128
    return np.ascontiguousarray(
        a.reshape(*lead, c, 128, last).swapaxes(-3, -2)
    )


def _perpart(v, scale=1.0):
    """[h, e] bias -> [128, h, EB] per-partition layout."""
    h, e = v.shape
    return np.ascontiguousarray(
        (np.asarray(v, np.float32) * scale).reshape(h, -1, 128).transpose(2, 0, 1)
    )


def _prep_inputs(Q, Wq, bq, Wk, bk, Wv, bv, Wo, bo):
    t, b, d = Q.shape
    h, e, _ = Wq.shape
    s = np.float32(1.0 / np.sqrt(e))
    bf = ml_dtypes.bfloat16
    Q = np.asarray(Q, np.float32)
    Wq = np.asarray(Wq, np.float32)
    Wk = np.asarray(Wk, np.float32)
    bk = np.asarray(bk, np.float32)
    # [B, 128, DC, T] partition-major Q^T per batch
    qt_all = _pmajor(Q.transpose(1, 2, 0).astype(bf), t)
    # scores kernel M = s * Wq^T Wk  [h, d, d] (bq terms cancel in the
    # query-axis softmax; the bk@q rank-1 term is sent separately)
    M = np.einsum("hed,hef->hdf", Wq, Wk, optimize=True) * s
    mqt = _pmajor(M.astype(bf), d)
    # wqg[b,h,t] = s*(Wq^T bk)@Q_t / 128 (the /128 compensates the ones-
    # column rank-1 matmul on device)
    w1 = np.einsum("hed,he->hd", Wq, bk, optimize=True)
    wqg = np.einsum("hd,tbd->bht", w1, Q, optimize=True) * (s / 128.0)
    wqg = np.ascontiguousarray(wqg.astype(bf))
    wvt = _pmajor(np.asarray(Wv, np.float32).transpose(0, 2, 1).astype(bf), e)
    wot = _pmajor(np.asarray(Wo, np.float32).T.reshape(h, e, d).astype(bf), d)
    shared = {
        "mqt": mqt,
        "wvt": wvt,
        "wot": wot,
        "bvs": _perpart(bv),
        "bos": np.ascontiguousarray(
            np.asarray(bo, np.float32).reshape(-1, 128).T
        ),
    }
    in_maps = [
        {"qt": np.ascontiguousarray(qt_all[bb]), "wqg": wqg[bb], **shared}
        for bb in range(b)
    ]
    return in_maps, (t, d, h, e)


def kernel(Q, Wq, bq, Wk, bk, Wv, bv, Wo, bo, _trace=False):
    in_maps, (t, d, h, e) = _prep_inputs(Q, Wq, bq, Wk, bk, Wv, bv, Wo, bo)
    nc = _get_nc((t, d, h, e))
    res = bass_utils.run_bass_kernel_spmd(
        nc, in_maps, core_ids=list(range(len(in_maps))), trace=_trace
    )
    # device output is [d, t]; transpose back per core
    out = np.stack(
        [res.results[b]["out"].T for b in range(len(in_maps))], axis=1
    )
    if _trace:
        kernel.last_results = res
    return np.ascontiguousarray(out.astype(np.float32))


# revision 27
# speedup vs baseline: 1.4301x; 1.0899x over previous
"""Multi-head attention (softmax over the QUERY axis) on 8 TRN2 NeuronCores.

Problem shapes: Q [T=1024, B=8, D=256]; per-head full-width projections
Wq/Wk/Wv [H=8, E=512, D=256]; Wo [D=256, H*E=4096].

Sharding: data-parallel over batch B — core b computes all H heads for
batch b. No collectives; the host re-stacks per-core outputs along B.

Both big einsum chains are factored through D=256 (< E=512), since
scores and output have rank <= D:
  scores: S = (Wq Q)^T (Wk Q) = Q^T M Q with M = s*Wq^T Wk  [D,D, host]
  output: out = sum_s E[s,t] W2[s,d] with W2 = (v @ Wo_h)/l
Softmax over the QUERY axis t makes all per-key-row constants cancel in
E/l, so the bq bias terms drop exactly; the surviving per-query term
s*(Wq^T bk)@Q_t is added to the scores as a rank-1 matmul (ones x
broadcast row, host-precomputed). All matmuls bf16, accum fp32.

Per-core, per-head:
  MQ[d',t] = M_h @ Q_b^T            (8 matmuls)
  AT[s,t]  = Q MQ + 1 x wqg         (16 tiles x (2+1) matmuls)
  E[s,t]   = exp(AT); l[s] = row-sum (fused accum_out)
  vT[e,t]  = Wv_h @ Q_b^T + bv
  W2[s,d]  = (vT^T-blocks x Wo_h^T) / l[s]
  out[d,t] += W2^T-blocks x E
Output leaves the device [d, t]-transposed; the host transposes back.
"""

import sys

sys.path.insert(0, "/opt/trn_rl_repo")

from contextlib import ExitStack

import ml_dtypes
import numpy as np

import concourse.bass as bass
import concourse.tile as tile
from concourse.tile import add_dep_helper
from concourse import bacc, bass_utils, mybir

T, B, D, H, E = 1024, 8, 256, 8, 512
N_CORES = 8

F32 = mybir.dt.float32
BF16 = mybir.dt.bfloat16
FP8 = mybir.dt.float8e4
DR = mybir.MatmulPerfMode.DoubleRow
AF = mybir.ActivationFunctionType

# M and the rank-1 row are scaled by SM host-side so MQ sits in fp8 range
# (unscaled MQ std ~0.021 is e4m3-subnormal); the Exp divides it back out.
SM = 512.0


def _bcast(ap_row, parts):
    """Partition-broadcast a [1, n] DRAM AP to [parts, n] (step-0 partition)."""
    return bass.AP(
        tensor=ap_row.tensor,
        offset=ap_row.offset,
        ap=[[0, parts], list(ap_row.ap[-1])],
    )


def build_nc(t=T, d=D, h=H, e=E):
    """Build the per-core SPMD program. Returns a compiled Bacc."""
    TC = t // 512   # t chunks (512-wide matmul free dim)
    SB = t // 128   # s blocks (keys == queries length)
    EB = e // 128   # e blocks
    DC = d // 128   # d chunks (contraction for projections / scores)
    DB = d // 128   # d blocks (partition tiles of the transposed out)

    nc = bacc.Bacc("TRN2", target_bir_lowering=False, debug=False)

    # All big inputs arrive pre-arranged partition-major ([128, free...])
    # so every load is a clean 2D DMA with one contiguous row per partition.
    qt_d = nc.dram_tensor("qt", [128, DC, t], BF16, kind="ExternalInput").ap()
    qt8_d = nc.dram_tensor("qt8", [128, DC, t], FP8, kind="ExternalInput").ap()
    mqt_d = nc.dram_tensor("mqt", [h, 128, DC, d], BF16, kind="ExternalInput").ap()
    wvt_d = nc.dram_tensor("wvt", [h, 128, DC, e], BF16, kind="ExternalInput").ap()
    wot_d = nc.dram_tensor("wot", [h, 128, EB, d], BF16, kind="ExternalInput").ap()
    wqg_d = nc.dram_tensor("wqg", [h, t], BF16, kind="ExternalInput").ap()
    bv_d = nc.dram_tensor("bvs", [128, h, EB], F32, kind="ExternalInput").ap()
    bo_d = nc.dram_tensor("bos", [128, DB], F32, kind="ExternalInput").ap()
    out_d = nc.dram_tensor("out", [d, t], F32, kind="ExternalOutput").ap()

    with tile.TileContext(nc) as tc, ExitStack() as ctx:
        consts = ctx.enter_context(tc.tile_pool(name="consts", bufs=1))
        wpool = ctx.enter_context(tc.tile_pool(name="wpool", bufs=2))
        hpool = ctx.enter_context(tc.tile_pool(name="hpool", bufs=2))
        spool = ctx.enter_context(tc.tile_pool(name="spool", bufs=2))
        at_pool = ctx.enter_context(tc.tile_pool(name="at_pool", bufs=3, space="PSUM"))
        mm_pool = ctx.enter_context(tc.tile_pool(name="mm_pool", bufs=5, space="PSUM"))

        # ---- persistent loads -------------------------------------------
        # Q^T: the dc0 chunk split in quarters across DMA queues so the
        # first matmul's inputs land ASAP.
        qt_sb = consts.tile([128, DC, t], BF16)
        for qtr in range(4):
            qsl = slice(qtr * (t // 4), (qtr + 1) * (t // 4))
            nc.sync.dma_start(out=qt_sb[:, 0, qsl], in_=qt_d[:, 0, qsl])
        bv_sb = consts.tile([128, h, EB], F32)
        nc.sync.dma_start(out=bv_sb, in_=bv_d)
        bo_sb = consts.tile([128, DB], F32)
        nc.gpsimd.dma_start(out=bo_sb, in_=bo_d)
        qt8_sb = consts.tile([128, DC, t], FP8)
        out_acc = consts.tile([128, DB, t], F32)
        out_r = out_d.rearrange("(db p) t -> p db t", p=128)

        # ones column for the rank-1 bias-term matmul (host divides by 128)
        ones_sb = consts.tile([128, 128], BF16)
        nc.vector.memset(ones_sb, 1.0)

        # ---- PE warm-up: dummy matmuls during the initial DMA wait so the
        # HAM clock-gate ramps before real work lands ----------------------
        scratch = consts.tile([128, 640], BF16)
        nc.vector.memset(scratch, 0.0)
        ps_w = mm_pool.tile([128, 512], F32, tag="mm")
        for _ in range(10):
            nc.tensor.matmul(
                ps_w, scratch[:, :128], scratch[:, 128:640], start=True, stop=True
            )

        # Per-head SBUF tile handles, filled by emit_loads/emit_mqv.
        mq_t, wv_t, wo_t, wqg_t = {}, {}, {}, {}
        MQ_t, vT_t = {}, {}
        gated = []  # head-0 bulk loads deferred past the first matmul
        first_mm = [None]

        def emit_loads(hh):
            mq_sb = wpool.tile([128, DC, d], BF16, name="mq")
            for dc in range(DC):
                nc.sync.dma_start(out=mq_sb[:, dc, :], in_=mqt_d[hh, :, dc, :])
            if hh == 0:
                # rest of Q^T after the critical M chunks
                for qtr in range(4):
                    qsl = slice(qtr * (t // 4), (qtr + 1) * (t // 4))
                    nc.sync.dma_start(out=qt_sb[:, 1, qsl], in_=qt_d[:, 1, qsl])
            wv_sb = wpool.tile([128, DC, e], BF16, name="wv")
            g1 = nc.sync.dma_start(out=wv_sb, in_=wvt_d[hh])
            wo_sb = wpool.tile([128, EB, d], BF16, name="wo")
            g2 = nc.sync.dma_start(out=wo_sb, in_=wot_d[hh])
            wqg_bc = wpool.tile([128, t], BF16, name="wqg")
            g3 = nc.gpsimd.dma_start(out=wqg_bc, in_=_bcast(wqg_d[hh][None, :], 128))
            if hh == 0:
                for dc in range(DC):
                    g4 = nc.sync.dma_start(out=qt8_sb[:, dc, :], in_=qt8_d[:, dc, :])
                    gated.append(g4)
                gated.extend([g1, g2, g3])
            mq_t[hh], wv_t[hh], wo_t[hh], wqg_t[hh] = mq_sb, wv_sb, wo_sb, wqg_bc

        def emit_mqv(hh):
            """MQ = M_h @ Q^T (bf16 out) and vT = Wv_h @ Q^T + bv."""
            mq_sb, wv_sb = mq_t[hh], wv_t[hh]
            MQ = hpool.tile([128, DC, t], FP8, name="MQ")
            vT = hpool.tile([128, EB, t], BF16, name="vT")
            for db in range(DB):
                for tch in range(TC):
                    tsl = slice(tch * 512, (tch + 1) * 512)
                    ps_m = mm_pool.tile([128, 512], F32, tag="mm", name="ps_m")
                    for dc in range(DC):
                        mm = nc.tensor.matmul(
                            ps_m,
                            mq_sb[:, dc, db * 128 : (db + 1) * 128],
                            qt_sb[:, dc, tsl],
                            start=(dc == 0),
                            stop=(dc == DC - 1),
                        )
                        if first_mm[0] is None:
                            first_mm[0] = mm
                    nc.vector.tensor_copy(MQ[:, db, tsl], ps_m)
            if hh == 0:
                for g in gated:
                    add_dep_helper(
                        g.ins, first_mm[0].ins,
                        reason="defer bulk load past cold start",
                    )
            for eb in range(EB):
                for tch in range(TC):
                    tsl = slice(tch * 512, (tch + 1) * 512)
                    ps_v = mm_pool.tile([128, 512], F32, tag="mm", name="ps_v")
                    for dc in range(DC):
                        nc.tensor.matmul(
                            ps_v,
                            wv_sb[:, dc, eb * 128 : (eb + 1) * 128],
                            qt_sb[:, dc, tsl],
                            start=(dc == 0),
                            stop=(dc == DC - 1),
                        )
                    nc.scalar.activation(
                        vT[:, eb, tsl],
                        ps_v,
                        AF.Identity,
                        bias=bv_sb[:, hh, eb : eb + 1],
                    )
            MQ_t[hh], vT_t[hh] = MQ, vT

        def emit_scores(hh):
            """AT = Q^T M Q + rank-1 bias term -> exp -> rowsum -> 1/l."""
            MQ, wqg_bc = MQ_t[hh], wqg_t[hh]
            Ex = hpool.tile([128, SB, t], BF16, name="Ex")
            lsum2 = spool.tile([128, SB, TC], F32, name="lsum2")
            lsum = spool.tile([128, SB], F32, name="lsum")
            rr = spool.tile([128, SB], F32, name="rr")
            for sb in range(SB):
                ssl = slice(sb * 128, (sb + 1) * 128)
                for tch in range(TC):
                    tsl = slice(tch * 512, (tch + 1) * 512)
                    at = at_pool.tile([128, 512], F32, tag="at", name="at")
                    nc.tensor.matmul(
                        at, ones_sb, wqg_bc[:, tsl], start=True, stop=False
                    )
                    nc.tensor.matmul(
                        at,
                        qt8_sb[:, 0:DC, ssl],
                        MQ[:, 0:DC, tsl],
                        start=False,
                        stop=True,
                        perf_mode=DR,
                    )
                    nc.scalar.activation(
                        Ex[:, sb, tsl],
                        at,
                        AF.Exp,
                        scale=float(1.0 / SM),
                        accum_out=lsum2[:, sb, tch : tch + 1],
                    )
                if TC == 1:
                    nc.vector.reciprocal(rr[:, sb : sb + 1], lsum2[:, sb, 0:1])
                else:
                    nc.vector.reduce_sum(
                        lsum[:, sb : sb + 1],
                        lsum2[:, sb, :],
                        axis=mybir.AxisListType.X,
                    )
                    nc.vector.reciprocal(rr[:, sb : sb + 1], lsum[:, sb : sb + 1])
            return Ex, rr

        def emit_w2_out(hh, Ex, rr):
            """W2 = (vT^T x Wo^T)/l, then out[d,t] += W2^T x E."""
            wo_sb = wo_t[hh]
            vT = vT_t[hh]
            W2 = hpool.tile([128, SB, d], BF16, name="W2")
            for sb in range(SB):
                ssl = slice(sb * 128, (sb + 1) * 128)
                ps_2 = mm_pool.tile([128, 512], F32, tag="mm", name="ps_2")
                for eb in range(EB):
                    nc.tensor.matmul(
                        ps_2[:, :d],
                        vT[:, eb, ssl],
                        wo_sb[:, eb, :],
                        start=(eb == 0),
                        stop=(eb == EB - 1),
                    )
                nc.vector.tensor_scalar_mul(
                    W2[:, sb, :], ps_2[:, :d], rr[:, sb : sb + 1]
                )
            for db in range(DB):
                dsl = slice(db * 128, (db + 1) * 128)
                for tch in range(TC):
                    tsl = slice(tch * 512, (tch + 1) * 512)
                    ps_f = mm_pool.tile([128, 512], F32, tag="mm", name="ps_f")
                    for sc in range(SB):
                        nc.tensor.matmul(
                            ps_f,
                            W2[:, sc, dsl],
                            Ex[:, sc, tsl],
                            start=(sc == 0),
                            stop=(sc == SB - 1),
                        )
                    if hh == 0:
                        nc.vector.tensor_scalar_add(
                            out_acc[:, db, tsl], ps_f, bo_sb[:, db : db + 1]
                        )
                    else:
                        nc.vector.tensor_add(
                            out_acc[:, db, tsl], out_acc[:, db, tsl], ps_f
                        )
                    if hh == h - 1:
                        nc.sync.dma_start(
                            out=out_r[:, db, tsl], in_=out_acc[:, db, tsl]
                        )

        # ---- head loop (tile scheduler handles cross-phase overlap) -----
        for hh in range(h):
            emit_loads(hh)
            emit_mqv(hh)
            ex_rr = emit_scores(hh)
            emit_w2_out(hh, *ex_rr)

    nc.compile()
    return nc


_NC_CACHE = {}


def _get_nc(shape_key):
    if shape_key not in _NC_CACHE:
        _NC_CACHE[shape_key] = build_nc(*shape_key)
    return _NC_CACHE[shape_key]


def _pmajor(a, last):
    """[..., C*128, last] -> [..., 128, C, last] partition-major layout."""
    lead = a.shape[:-2]
    c = a.shape[-2] // 128
    return np.ascontiguousarray(
        a.reshape(*lead, c, 128, last).swapaxes(-3, -2)
    )


def _perpart(v, scale=1.0):
    """[h, e] bias -> [128, h, EB] per-partition layout."""
    h, e = v.shape
    return np.ascontiguousarray(
        (np.asarray(v, np.float32) * scale).reshape(h, -1, 128).transpose(2, 0, 1)
    )


def _prep_inputs(Q, Wq, bq, Wk, bk, Wv, bv, Wo, bo):
    t, b, d = Q.shape
    h, e, _ = Wq.shape
    s = np.float32(1.0 / np.sqrt(e))
    bf = ml_dtypes.bfloat16
    Q = np.asarray(Q, np.float32)
    Wq = np.asarray(Wq, np.float32)
    Wk = np.asarray(Wk, np.float32)
    bk = np.asarray(bk, np.float32)
    # [B, 128, DC, T] partition-major Q^T per batch
    qt_all = _pmajor(Q.transpose(1, 2, 0).astype(bf), t)
    f8 = ml_dtypes.float8_e4m3
    qt8_all = _pmajor(Q.transpose(1, 2, 0).astype(f8), t)
    # scores kernel M = s * Wq^T Wk  [h, d, d] (bq terms cancel in the
    # query-axis softmax; the bk@q rank-1 term is sent separately)
    M = np.einsum("hed,hef->hdf", Wq, Wk, optimize=True) * (s * SM)
    mqt = _pmajor(M.astype(bf), d)
    # wqg[b,h,t] = s*(Wq^T bk)@Q_t / 128 (the /128 compensates the ones-
    # column rank-1 matmul on device)
    w1 = np.einsum("hed,he->hd", Wq, bk, optimize=True)
    wqg = np.einsum("hd,tbd->bht", w1, Q, optimize=True) * (s * SM / 128.0)
    wqg = np.ascontiguousarray(wqg.astype(bf))
    wvt = _pmajor(np.asarray(Wv, np.float32).transpose(0, 2, 1).astype(bf), e)
    wot = _pmajor(np.asarray(Wo, np.float32).T.reshape(h, e, d).astype(bf), d)
    shared = {
        "mqt": mqt,
        "wvt": wvt,
        "wot": wot,
        "bvs": _perpart(bv),
        "bos": np.ascontiguousarray(
            np.asarray(bo, np.float32).reshape(-1, 128).T
        ),
    }
    in_maps = [
        {"qt": np.ascontiguousarray(qt_all[bb]),
         "qt8": np.ascontiguousarray(qt8_all[bb]), "wqg": wqg[bb], **shared}
        for bb in range(b)
    ]
    return in_maps, (t, d, h, e)


def kernel(Q, Wq, bq, Wk, bk, Wv, bv, Wo, bo, _trace=False):
    in_maps, (t, d, h, e) = _prep_inputs(Q, Wq, bq, Wk, bk, Wv, bv, Wo, bo)
    nc = _get_nc((t, d, h, e))
    res = bass_utils.run_bass_kernel_spmd(
        nc, in_maps, core_ids=list(range(len(in_maps))), trace=_trace
    )
    # device output is [d, t]; transpose back per core
    out = np.stack(
        [res.results[b]["out"].T for b in range(len(in_maps))], axis=1
    )
    if _trace:
        kernel.last_results = res
    return np.ascontiguousarray(out.astype(np.float32))
